# revision 69
# baseline (speedup 1.0000x reference)
"""GCN message-passing kernel for Trainium2 (8 NeuronCores, SPMD).

Math (matches the reference):
    gf   = RF @ W_g                          (2048, 3)   gate features
    H_k  = RF @ W_k                          (2048, 4096) per edge type k in {0,1,2}
    gate(e) = sigmoid(gf[src_e, k_e] + b_glab[p_e])
    upd[t]  = sum_{e->t} gate(e) * (H_{k_e}[src_e] + b_lab[p_e])
    out  = relu(upd)

Key FLOP reduction vs computing H_k for all regions x 3 edge types
(48 block-units of 128x4096x512 matmul per core): edge types 0/1 only
need H at rows that are message SOURCES.  Across the batch only ~940
distinct (row, k) sources exist per edge type, so the host packs the
distinct source rows of each group of 8 images into one 128-row "pack
block" (16 pack blocks total, 8 per edge type).  The device computes
    H2 (self-loop)  : 16 region blocks   (g2-gate folded at PSUM copy)
    Hpack           : 16 pack blocks
i.e. 32 big-matmul block-units instead of 48.  Gates / the
block-diagonal scatter matrices are built ON DEVICE from gf with
one-hot constant matrices (host only prepares 0/1 index matrices and
row permutations), so all data-dependent FLOPs run on Trainium.
gf rides inside the H2 matmul stream via ldweights sharing.

Sharding: the output D dim (4096) is split 8 ways -> each core computes
all 2048 rows x its 512 columns, holding a (4096 x 3*512) slice of
W_conv.  No collectives; host concatenates the column slices.
"""

import numpy as np
import ml_dtypes

# problem constants (hardcoded per contract)
N_IMG = 64
REG = 32
RPI = 32
NUM_REL = 20
D = 4096
NPRED = 81
N = N_IMG * REG          # 2048
NCORES = 8
CW = D // NCORES         # 512 output cols per core
NBLK = N // 128          # 16 row blocks
IPB = 128 // REG         # 4 images per block
EPB = IPB * NUM_REL      # 80 edges per block per edge type
NPK = 8                  # pack blocks per edge type (8 images each)
IPP = N_IMG // NPK       # 8 images per pack block

BF = ml_dtypes.bfloat16

_prog_cache = {}


def _build_program():
    import concourse.bass as bass
    import concourse.tile as tile
    from concourse import bacc, mybir

    bf16 = mybir.dt.bfloat16
    f32 = mybir.dt.float32
    AF = mybir.ActivationFunctionType
    ALU = mybir.AluOpType

    nc = bacc.Bacc("TRN2", target_bir_lowering=False, debug=False,
                   num_devices=NCORES)

    rft = nc.dram_tensor("rft", [NBLK, 128, 32 * 128], bf16, kind="ExternalInput").ap()
    pk = nc.dram_tensor("pk", [2 * NPK, 128, 32 * 128], bf16, kind="ExternalInput").ap()
    w = nc.dram_tensor("w", [128, 3 * 32 * CW], bf16, kind="ExternalInput").ap()
    wg = nc.dram_tensor("wg", [128, 32 * 3], bf16, kind="ExternalInput").ap()
    blab = nc.dram_tensor("blab", [NPRED, CW], bf16, kind="ExternalInput").ap()
    bgb = nc.dram_tensor("bgb", [128, NPRED], bf16, kind="ExternalInput").ap()
    srct = nc.dram_tensor("srct", [128, NBLK * 2 * EPB], bf16, kind="ExternalInput").ap()
    srcp = nc.dram_tensor("srcp", [EPB, NBLK * 2 * 128], bf16, kind="ExternalInput").ap()
    tgto = nc.dram_tensor("tgto", [EPB, NBLK * 2 * 128], bf16, kind="ExternalInput").ap()
    p1h = nc.dram_tensor("p1h", [EPB, NBLK * NPRED], bf16, kind="ExternalInput").ap()
    p1hs = nc.dram_tensor("p1hs", [128, NPRED], bf16, kind="ExternalInput").ap()
    ident = nc.dram_tensor("ident", [128, 128], bf16, kind="ExternalInput").ap()
    out = nc.dram_tensor("out", [NBLK, 128, CW], bf16, kind="ExternalOutput").ap()

    DEPTH_A = 4  # H2 blocks in the startup phase (absorbs w0/pk0 DMA time)
    with tile.TileContext(nc) as tc:
        with (
            tc.tile_pool(name="consts", bufs=1) as cpool,
            tc.tile_pool(name="rft", bufs=4) as rpool,
            tc.tile_pool(name="pk", bufs=4) as ppool,
            tc.tile_pool(name="deep", bufs=5) as dpool,
            tc.tile_pool(name="small", bufs=2) as spool,
            tc.tile_pool(name="osb", bufs=2) as opool,
            tc.tile_pool(name="ph", bufs=2, space="PSUM") as php,
            tc.tile_pool(name="pgf", bufs=1, space="PSUM") as pgfp,
            tc.tile_pool(name="prg", bufs=1, space="PSUM") as prgp,
            tc.tile_pool(name="pgt", bufs=1, space="PSUM") as pgtp,
            tc.tile_pool(name="pms", bufs=1, space="PSUM") as pmsp,
            tc.tile_pool(name="pout", bufs=2, space="PSUM") as poutp,
        ):
            # --- input DMAs.  Everything on the sync HW-DGE queue, in the
            # exact order the PE consumes it (one queue: arrival time is
            # cumulative-bytes / HBM bandwidth, so order is everything).
            # rft_tiles[b] = (tiles, d_per_tile)
            rft_tiles, pk_tiles = {}, {}

            def _load_chunk(pool, dram, b, j, dper, pfx, eng=None):
                t = pool.tile([128, dper * 128], bf16, tag=f"{pfx}{j}",
                              name=f"{pfx}{j}_{b}")
                (eng or nc.sync).dma_start(
                    out=t[:],
                    in_=dram[b, :, j * dper * 128:(j + 1) * dper * 128])
                return t

            def _load_rft(b):
                rft_tiles[b] = ([_load_chunk(rpool, rft, b, 0, 16, "rf"),
                                 _load_chunk(rpool, rft, b, 1, 16, "rf")], 16)

            def _load_pk(qq):
                pk_tiles[qq] = [_load_chunk(ppool, pk, qq, 0, 16, "pk"),
                                _load_chunk(ppool, pk, qq, 1, 16, "pk")]

            # first H2 pass needs wg + rft0.h0 + w2 chunk 0 first
            WCH = 4 * CW  # w2 chunk: 4 d-tiles
            w2_ch = [cpool.tile([128, WCH], bf16, tag=f"w2c{i}", name=f"w2c{i}")
                     for i in range(8)]
            w_sb_k = []
            for k in range(2):
                wk = cpool.tile([128, 32 * CW], bf16, tag=f"w{k}")
                w_sb_k.append(wk)

            def _load_w2c(i, eng=None):
                (eng or nc.sync).dma_start(
                    out=w2_ch[i][:],
                    in_=w[:, (2 * 32 + i * 4) * CW:(2 * 32 + (i + 1) * 4) * CW])

            # cold start: split the first-pass-critical ~5.5MB across the
            # two HW-DGE queues (sync + scalar) so queue ramp-up and the
            # first transfers run in parallel
            _rf0h0 = _load_chunk(rpool, rft, 0, 0, 16, "rf")
            wg_sb = cpool.tile([128, 32 * 3], bf16, tag="wg")
            nc.sync.dma_start(out=wg_sb[:], in_=wg[:])
            bgb_sb = cpool.tile([128, NPRED], bf16, tag="bgb")
            nc.sync.dma_start(out=bgb_sb[:], in_=bgb[:])
            _load_w2c(0)
            _load_w2c(1)
            _rf0h1 = _load_chunk(rpool, rft, 0, 1, 16, "rf")
            rft_tiles[0] = ([_rf0h0, _rf0h1], 16)
            _load_w2c(2)
            _load_w2c(3)
            _load_w2c(4)
            _load_w2c(5)
            _load_w2c(6)
            _load_w2c(7)
            _load_rft(1)
            _load_rft(2)
            _load_rft(3)
            nc.sync.dma_start(out=w_sb_k[0][:], in_=w[:, 0:32 * CW])
            _load_pk(0)          # k=0, q=0
            _load_rft(4)
            # small consts (needed by build(0))
            srct_sb = cpool.tile([128, NBLK * 2 * EPB], bf16, tag="srct")
            nc.sync.dma_start(out=srct_sb[:], in_=srct[:])
            srcp_sb = cpool.tile([EPB, NBLK * 2 * 128], bf16, tag="srcp")
            nc.sync.dma_start(out=srcp_sb[:], in_=srcp[:])
            tgto_sb = cpool.tile([EPB, NBLK * 2 * 128], bf16, tag="tgto")
            nc.sync.dma_start(out=tgto_sb[:], in_=tgto[:])
            p1h_sb = cpool.tile([EPB, NBLK * NPRED], bf16, tag="p1h")
            nc.sync.dma_start(out=p1h_sb[:], in_=p1h[:])
            _load_pk(NPK)        # k=1, q=0
            p1hs_sb = cpool.tile([128, NPRED], bf16, tag="p1hs")
            nc.sync.dma_start(out=p1hs_sb[:], in_=p1hs[:])
            ident_sb = cpool.tile([128, 128], bf16, tag="ident")
            nc.sync.dma_start(out=ident_sb[:], in_=ident[:])
            blab_sb = cpool.tile([NPRED, CW], bf16, tag="blab")
            nc.sync.dma_start(out=blab_sb[:], in_=blab[:])
            nc.sync.dma_start(out=w_sb_k[1][:], in_=w[:, 32 * CW:2 * 32 * CW])
            _load_rft(5)
            _load_pk(1)
            _load_pk(NPK + 1)

            gf_tiles, g2_tiles, h2_tiles, hp_tiles, mtgt = {}, {}, {}, {}, {}
            sig_tiles = {}

            def presig(b):
                """Precompute build(b)'s sig tiles while the scalar engine
                is idle (phase-A blocks only; their builds would otherwise
                wait on the sigmoid -> prg -> pg chain)."""
                gf_sb = gf_tiles[b]
                sig = []
                for k in range(2):
                    sg = dpool.tile([128, NPRED], bf16, tag=f"esig{k}",
                                    name=f"esig{b}_{k}")
                    nc.scalar.activation(sg[:], bgb_sb[:], AF.Sigmoid,
                                         bias=gf_sb[:, k:k + 1])
                    sig.append(sg)
                sig_tiles[b] = sig

            def rft_lhsT(b, d):
                tiles, dper = rft_tiles[b]
                return tiles[d // dper][:, (d % dper) * 128:(d % dper + 1) * 128]

            def pk_lhsT(qq, d):
                return pk_tiles[qq][d // 16][:, (d % 16) * 128:(d % 16 + 1) * 128]

            from concourse.tile_rust import add_dep_helper

            def _gf_post(b, ph_t, pgf_ap):
                """gf PSUM -> gates: gf copy, g2, h2s scale, sig tiles.
                Issued right after the pass so the scalar/vector chains
                overlap the next big pass instead of stalling build(b)."""
                gf_sb = dpool.tile([128, 3], f32, tag="gf", name=f"gf{b}")
                nc.vector.tensor_copy(out=gf_sb[:], in_=pgf_ap)
                gf_tiles[b] = gf_sb
                g2 = dpool.tile([128, 1], f32, tag="g2", name=f"g2_{b}")
                nc.scalar.activation(g2[:], bgb_sb[:, 0:1], AF.Sigmoid,
                                     bias=gf_sb[:, 2:3])
                g2_tiles[b] = g2
                h2s = dpool.tile([128, CW], bf16, tag="h2", name=f"h2_{b}")
                nc.vector.tensor_scalar_mul(h2s[:], ph_t[:], g2[:])
                h2_tiles[b] = h2s
                del rft_tiles[b]

            def _h2gf_mm(b, d, ph_t, pgf_ap, prev):
                lhsT = rft_lhsT(b, d)
                nc.tensor.matmul(ph_t[:], lhsT,
                                 w2_ch[d // 4][:, (d % 4) * CW:(d % 4 + 1) * CW],
                                 start=(d == 0), stop=(d == 31))
                h_inst = nc.main_func.blocks[-1].instructions[-1]
                assert h_inst.opcode == "Matmult"
                if prev is not None:
                    add_dep_helper(h_inst, prev, sync=False, reason="h2-chain")
                nc.tensor.matmul(pgf_ap, lhsT,
                                 wg_sb[:, d * 3:(d + 1) * 3],
                                 start=(d == 0), stop=(d == 31))
                gf_inst = nc.main_func.blocks[-1].instructions[-1]
                assert gf_inst.opcode == "Matmult"
                gf_inst.ldweights = False
                add_dep_helper(gf_inst, h_inst, sync=False, reason="h2-pair")
                return gf_inst

            def h2gf_pass(b):
                """H_2(b) = RF_b @ W_2 with gf(b) interleaved; the gf
                matmul reuses the H matmul's stationary operand via
                ldweights=False.  g2 is folded into the PSUM->SBUF copy:
                h2s = diag(g2) @ H_2."""
                ph_t = php.tile([128, CW], f32, tag="ph", name=f"ph2_{b}")
                pgf_t = pgfp.tile([128, 3], f32, tag="pgf", name=f"pgf{b}")
                prev = None
                for d in range(32):
                    prev = _h2gf_mm(b, d, ph_t, pgf_t[:], prev)
                _gf_post(b, ph_t, pgf_t[:])



            def hpack_pass(q, k):
                """Hpack(q,k) = PK_{k,q} @ W_k  (distinct source rows)."""
                qq = k * NPK + q
                ph_t = php.tile([128, CW], f32, tag="ph", name=f"php{qq}")
                for d in range(32):
                    nc.tensor.matmul(ph_t[:], pk_lhsT(qq, d),
                                     w_sb_k[k][:, d * CW:(d + 1) * CW],
                                     start=(d == 0), stop=(d == 31))
                hp = dpool.tile([128, CW], bf16, tag="hp", name=f"hp{qq}")
                nc.vector.tensor_copy(out=hp[:], in_=ph_t[:])
                hp_tiles[(q, k)] = hp
                del pk_tiles[qq]

            bld_state = {}

            def build_a(b):
                """Trigger half of the gate build: prg matmuls + the
                vector chain (pg/gcol/srcg).  Separating this from the
                consumer matmuls (build_b) lets a big pass run between
                them so the tensor engine never waits on vector."""
                gf_sb = gf_tiles[b]
                if b in sig_tiles:
                    sig = sig_tiles.pop(b)
                else:
                    sig = []
                    for k in range(2):
                        sg = spool.tile([128, NPRED], bf16, tag=f"sig{k}",
                                        name=f"sig{b}_{k}")
                        nc.scalar.activation(sg[:], bgb_sb[:], AF.Sigmoid,
                                             bias=gf_sb[:, k:k + 1])
                        sig.append(sg)
                pgs, srcgs = [], []
                for k in range(2):
                    prg_t = prgp.tile([EPB, NPRED], f32, tag="prg",
                                      name=f"prg{b}_{k}")
                    nc.tensor.matmul(
                        prg_t[:],
                        srct_sb[:, (b * 2 + k) * EPB:(b * 2 + k + 1) * EPB],
                        sig[k][:], start=True, stop=True)
                    pg = spool.tile([EPB, NPRED], bf16, tag="pg",
                                    name=f"pg{b}_{k}")
                    nc.vector.tensor_mul(
                        pg[:], prg_t[:],
                        p1h_sb[:, b * NPRED:(b + 1) * NPRED])
                    gcol = spool.tile([EPB, 1], f32, tag="gcol",
                                      name=f"gcol{b}_{k}")
                    nc.vector.tensor_reduce(gcol[:], pg[:],
                                            axis=mybir.AxisListType.X,
                                            op=ALU.add)
                    srcg = spool.tile([EPB, 128], bf16, tag="srcg",
                                      name=f"srcg{b}_{k}")
                    nc.vector.tensor_scalar_mul(
                        srcg[:],
                        srcp_sb[:, (b * 2 + k) * 128:(b * 2 + k + 1) * 128],
                        gcol[:])
                    pgs.append(pg)
                    srcgs.append(srcg)
                bld_state[b] = (pgs, srcgs)

            def build_b(b):
                """Consumer half: pgt/pms matmuls + msel/gt copies."""
                pgs, srcgs = bld_state.pop(b)
                msel_sb = dpool.tile([128, 2 * 128], bf16, tag="msel",
                                     name=f"msel{b}")
                pgt_t = pgtp.tile([NPRED, 128], f32, tag="pgt", name=f"pgt{b}")
                for k in range(2):
                    nc.tensor.matmul(
                        pgt_t[:], pgs[k][:],
                        tgto_sb[:, (b * 2 + k) * 128:(b * 2 + k + 1) * 128],
                        start=(k == 0), stop=False)
                    pms_t = pmsp.tile([128, 128], f32, tag="pms",
                                      name=f"pms{b}_{k}")
                    nc.tensor.matmul(
                        pms_t[:], srcgs[k][:],
                        tgto_sb[:, (b * 2 + k) * 128:(b * 2 + k + 1) * 128],
                        start=True, stop=True)
                    nc.vector.tensor_copy(out=msel_sb[:, k * 128:(k + 1) * 128],
                                          in_=pms_t[:])
                # self-loop: G row 0 += g2
                pg2 = spool.tile([128, NPRED], bf16, tag="pg2", name=f"pg2_{b}")
                nc.vector.tensor_scalar_mul(pg2[:], p1hs_sb[:], g2_tiles[b][:])
                nc.tensor.matmul(pgt_t[:], pg2[:], ident_sb[:],
                                 start=False, stop=True)
                gt_sb = dpool.tile([NPRED, 128], bf16, tag="gt", name=f"gt{b}")
                nc.vector.tensor_copy(out=gt_sb[:], in_=pgt_t[:])
                mtgt[b] = (msel_sb, gt_sb)

            def build(b):
                build_a(b)
                build_b(b)

            pout_open = {}

            def stage3a(b):
                """First half of the output accumulation: k=0 scatter + G
                matmul (everything not needing Hpack(q,1))."""
                msel_sb, gt_sb = mtgt[b]
                q = b // 2
                pout_t = poutp.tile([128, CW], f32, tag="pout", name=f"po{b}")
                nc.tensor.matmul(pout_t[:], msel_sb[:, 0:128],
                                 hp_tiles[(q, 0)][:], start=True, stop=False)
                nc.tensor.matmul(pout_t[:], gt_sb[:], blab_sb[:],
                                 start=False, stop=False)
                pout_open[b] = pout_t

            def stage3b(b, mm_add=False):
                msel_sb, gt_sb = mtgt[b]
                q = b // 2
                pout_t = pout_open.pop(b)
                nc.tensor.matmul(pout_t[:], msel_sb[:, 128:256],
                                 hp_tiles[(q, 1)][:], start=False,
                                 stop=not mm_add)
                if mm_add:
                    # fold the h2s add into the accumulation group via an
                    # identity matmul: shortens the post-last-pass tail
                    nc.tensor.matmul(pout_t[:], ident_sb[:], h2_tiles[b][:],
                                     start=False, stop=True)
                else:
                    nc.vector.tensor_add(pout_t[:], pout_t[:],
                                         h2_tiles[b][:])
                out_sb = opool.tile([128, CW], bf16, tag="out", name=f"ob{b}")
                nc.scalar.activation(out_sb[:], pout_t[:], AF.Relu)
                nc.sync.dma_start(out=out[b], in_=out_sb[:])
                del gf_tiles[b], g2_tiles[b], h2_tiles[b], mtgt[b]
                if b % 2 == 1:
                    del hp_tiles[(q, 0)], hp_tiles[(q, 1)]

            def stage3(b):
                stage3a(b)
                stage3b(b)

            # --- phase A: H2 for the first blocks while w0/w1/pk stream ---
            for b in range(DEPTH_A):
                h2gf_pass(b)
            for b in range(DEPTH_A):
                presig(b)

            # --- steady state over pack-block groups; build/stage3 spread
            # between the big passes so machinery hides under them and the
            # tail after the last pass is short.  H2 blocks 4..11 run two
            # per iter (q<4), 12..15 one per iter (q>=4) so the last iters
            # still have big passes covering their build/stage3 machinery ---
            for q in range(NPK):
                hpack_pass(q, 0)
                if q < 2:
                    # congested startup window: split builds so their
                    # vector chains complete under the big passes
                    h2gf_pass(2 * q + DEPTH_A)
                    if 2 * q + 6 < 12:
                        _load_rft(2 * q + 6)
                        _load_rft(2 * q + 7)
                    build_a(2 * q)
                    hpack_pass(q, 1)
                    if q + 2 < NPK:
                        _load_pk(q + 2)
                        _load_pk(NPK + q + 2)
                    build_b(2 * q)
                    build_a(2 * q + 1)
                    stage3(2 * q)
                    h2gf_pass(2 * q + DEPTH_A + 1)
                    build_b(2 * q + 1)
                    stage3(2 * q + 1)
                elif q < 4:
                    h2gf_pass(2 * q + DEPTH_A)
                    if 2 * q + 6 < 12:
                        _load_rft(2 * q + 6)
                        _load_rft(2 * q + 7)
                    build(2 * q)
                    hpack_pass(q, 1)
                    if q + 2 < NPK:
                        _load_pk(q + 2)
                        _load_pk(NPK + q + 2)
                    build(2 * q + 1)
                    stage3(2 * q)
                    h2gf_pass(2 * q + DEPTH_A + 1)
                    stage3(2 * q + 1)
                    if q == 3:
                        _load_rft(12)
                elif q < NPK - 1:
                    build(2 * q)
                    h2gf_pass(q + 8)
                    if q + 9 < NBLK:
                        _load_rft(q + 9)
                    if q + 2 < NPK:
                        _load_pk(q + 2)
                        _load_pk(NPK + q + 2)
                    build(2 * q + 1)
                    hpack_pass(q, 1)
                    stage3(2 * q)
                    stage3(2 * q + 1)
                else:
                    # last iter: everything that can precede the final big
                    # pass does, so only one matmul + add/relu trail it
                    build(2 * q)
                    h2gf_pass(q + 8)
                    build(2 * q + 1)
                    stage3a(2 * q)
                    stage3a(2 * q + 1)
                    hpack_pass(q, 1)
                    stage3b(2 * q, mm_add=True)
                    stage3b(2 * q + 1, mm_add=True)

    nc.compile()
    return nc


def _pack_maps(rels):
    """Slot assignment: pack block q (per edge type k) holds the distinct
    source rows of images [8q, 8q+8).  Returns (ok, rows[k][q] global row
    lists, slot_of[k] dict region->(q, slot))."""
    rels_r = np.asarray(rels).reshape(N_IMG, RPI, 3)[:, :NUM_REL].reshape(-1, 3)
    im = rels_r[:, 0]
    rows = [[None] * NPK for _ in range(2)]
    slot_of = [{}, {}]
    for k, src in [(0, rels_r[:, 2]), (1, rels_r[:, 1])]:
        for q in range(NPK):
            m = (im >= q * IPP) & (im < (q + 1) * IPP)
            r = np.unique(src[m])
            if len(r) > 128:
                return False, None, None
            rows[k][q] = r
            for j, rr in enumerate(r):
                slot_of[k][int(rr)] = (q, j)
    return True, rows, slot_of


def _host_prep(inputs, pack):
    rows, slot_of = pack
    rf = np.asarray(inputs["region_feats"], dtype=np.float32)
    W = np.asarray(inputs["W_conv"], dtype=np.float32)
    Wg = np.asarray(inputs["W_g"], dtype=np.float32)
    blab = np.asarray(inputs["b_lab"], dtype=np.float32)
    bglab = np.asarray(inputs["b_glab"], dtype=np.float32)
    rels = np.asarray(inputs["rels"])
    preds = np.asarray(inputs["pred_classes"])

    rels_r = rels.reshape(N_IMG, RPI, 3)[:, :NUM_REL].reshape(-1, 3)
    preds_r = preds.reshape(N_IMG, RPI)[:, :NUM_REL].reshape(-1)

    rf_bf = rf.astype(BF)
    # RF^T tiles: rft_h[b, p, d*128+j] = RF[b*128+j, d*128+p]
    rft_h = np.ascontiguousarray(
        rf_bf.T.reshape(32, 128, NBLK, 128).transpose(2, 1, 0, 3)
    ).reshape(NBLK, 128, 32 * 128)

    # pack blocks: pk_h[k*8+q, p, d*128+j] = RF[rows[k][q][j], d*128+p]
    pk_rows = np.zeros((2 * NPK, 128, D), np.float32)
    for k in range(2):
        for q in range(NPK):
            r = rows[k][q]
            pk_rows[k * NPK + q, :len(r)] = rf[r]
    pk_h = np.ascontiguousarray(
        pk_rows.astype(BF).transpose(0, 2, 1).reshape(2 * NPK, 32, 128, 128)
        .transpose(0, 2, 1, 3)
    ).reshape(2 * NPK, 128, 32 * 128)

    # W slices per core: w_h[p, ((k*32+d)*CW)+j] = W[d*128+p, k*D + c*CW + j]
    Wr = W.reshape(32, 128, 3, NCORES, CW)
    w_cores = [
        np.ascontiguousarray(Wr[:, :, :, c, :].transpose(1, 2, 0, 3),
                             dtype=BF).reshape(128, 3 * 32 * CW)
        for c in range(NCORES)
    ]
    wg_h = np.ascontiguousarray(
        Wg.reshape(32, 128, 3).transpose(1, 0, 2), dtype=BF
    ).reshape(128, 32 * 3)
    blab_cores = [
        np.ascontiguousarray(blab[:, c * CW:(c + 1) * CW], dtype=BF)
        for c in range(NCORES)
    ]
    bgb_h = np.ascontiguousarray(
        np.repeat(bglab.reshape(1, NPRED), 128, axis=0), dtype=BF)

    srct_h = np.zeros((128, NBLK * 2 * EPB), np.float32)
    srcp_h = np.zeros((EPB, NBLK * 2 * 128), np.float32)
    tgto_h = np.zeros((EPB, NBLK * 2 * 128), np.float32)
    p1h_h = np.zeros((EPB, NBLK * NPRED), np.float32)
    e = np.arange(EPB)
    for b in range(NBLK):
        eb = rels_r[b * EPB:(b + 1) * EPB]
        pb = preds_r[b * EPB:(b + 1) * EPB]
        s = eb[:, 1] - b * 128
        o = eb[:, 2] - b * 128
        # k=0: obj -> subj (src=o, tgt=s); k=1: subj -> obj (src=s, tgt=o)
        srct_h[o, (b * 2 + 0) * EPB + e] = 1.0
        srct_h[s, (b * 2 + 1) * EPB + e] = 1.0
        tgto_h[e, (b * 2 + 0) * 128 + s] = 1.0
        tgto_h[e, (b * 2 + 1) * 128 + o] = 1.0
        p1h_h[e, b * NPRED + pb] = 1.0
        for k, src_loc in ((0, o), (1, s)):
            for ei in range(EPB):
                gsrc = int(src_loc[ei] + b * 128)
                q, j = slot_of[k][gsrc]
                assert q == b // 2, "pack block mismatch"
                srcp_h[ei, (b * 2 + k) * 128 + j] = 1.0
    p1hs_h = np.zeros((128, NPRED), np.float32)
    p1hs_h[:, 0] = 1.0

    shared = {
        "rft": rft_h,
        "pk": pk_h,
        "wg": wg_h,
        "bgb": bgb_h,
        "srct": srct_h.astype(BF),
        "srcp": srcp_h.astype(BF),
        "tgto": tgto_h.astype(BF),
        "p1h": p1h_h.astype(BF),
        "p1hs": p1hs_h.astype(BF),
        "ident": np.eye(128, dtype=np.float32).astype(BF),
    }
    in_maps = []
    for c in range(NCORES):
        m = dict(shared)
        m["w"] = w_cores[c]
        m["blab"] = blab_cores[c]
        in_maps.append(m)
    return in_maps


def _rels_are_blocked(rels):
    """Check each image's relations reference only that image's regions."""
    rels = np.asarray(rels)
    if rels.shape != (N_IMG * RPI, 3):
        return False
    rels_r = rels.reshape(N_IMG, RPI, 3)[:, :NUM_REL]
    img = np.arange(N_IMG)[:, None]
    lo, hi = img * REG, (img + 1) * REG
    so = rels_r[:, :, 1:3]
    if not np.all((so >= lo[:, :, None]) & (so < hi[:, :, None])):
        return False
    im_ok = np.all(rels_r[:, :, 0] == img)
    return bool(im_ok)


def _numpy_fallback(inputs):
    """Reference-equivalent host computation (only used if the per-image
    relation structure assumption is violated)."""
    rf = np.asarray(inputs["region_feats"], dtype=np.float32)
    W = np.asarray(inputs["W_conv"], dtype=np.float32)
    Wg = np.asarray(inputs["W_g"], dtype=np.float32)
    blab = np.asarray(inputs["b_lab"], dtype=np.float32)
    bglab = np.asarray(inputs["b_glab"], dtype=np.float32)
    rels = np.asarray(inputs["rels"])
    preds = np.asarray(inputs["pred_classes"])
    rels_r = rels.reshape(N_IMG, RPI, 3)[:, :NUM_REL].reshape(-1, 3)
    preds_r = preds.reshape(N_IMG, RPI)[:, :NUM_REL].reshape(-1)
    nf = (rf @ W).reshape(-1, D)
    gfe = (rf @ Wg).reshape(-1)
    s, o = rels_r[:, 1], rels_r[:, 2]
    self_ids = np.arange(N)
    idx = np.concatenate([o * 3 + 0, s * 3 + 1, self_ids * 3 + 2])
    pr = np.concatenate([preds_r, preds_r, np.zeros(N, preds_r.dtype)])
    tgt = np.concatenate([s, o, self_ids])
    gate = 1.0 / (1.0 + np.exp(-(gfe[idx] + bglab[pr, 0])))
    msg = gate[:, None] * (nf[idx] + blab[pr])
    upd = np.zeros((N, D), np.float32)
    np.add.at(upd, tgt, msg)
    return np.maximum(upd, 0.0)


def _run(inputs, trace=False):
    from concourse.bass_utils import run_bass_kernel_spmd

    ok, rows, slot_of = _pack_maps(inputs["rels"])
    assert ok
    if "nc" not in _prog_cache:
        _prog_cache["nc"] = _build_program()
    nc = _prog_cache["nc"]
    in_maps = _host_prep(inputs, (rows, slot_of))
    try:
        res = run_bass_kernel_spmd(nc, in_maps, core_ids=list(range(NCORES)),
                                   trace=trace)
    except Exception:
        # transient device errors (e.g. NRT_EXEC_UNIT_UNRECOVERABLE) have
        # been observed to clear on retry
        import time
        time.sleep(5)
        res = run_bass_kernel_spmd(nc, in_maps, core_ids=list(range(NCORES)),
                                   trace=trace)
    out = np.empty((N, D), np.float32)
    for c in range(NCORES):
        out[:, c * CW:(c + 1) * CW] = (
            np.asarray(res.results[c]["out"]).astype(np.float32).reshape(N, CW))
    return out, res


def kernel(**inputs):
    if not _rels_are_blocked(inputs["rels"]):
        return _numpy_fallback(inputs)
    ok, _, _ = _pack_maps(inputs["rels"])
    if not ok:
        return _numpy_fallback(inputs)
    out, _ = _run(inputs, trace=False)
    return out
